# revision 28
# baseline (speedup 1.0000x reference)
"""Trainium2 Bass kernel for nn_ConceptIntergation (histogram_binning).

Reference computation:
    counts[b,s,n] = sum_k one_hot(concepts[b,s,k], 129)[..., n]  (n < 128; 128 = padding)
    out[b,s,n,d]  = counts[b,s,n] * emb_table[n,d]

Strategy (data-parallel over batch, 8 cores):
  - Each core handles B_LOC=8 batches -> 1600 (b,s) rows, output shard
    [1600, 128*64] f32 (~52 MB). The kernel is HBM-write bound; the design
    keeps the 16 SDMA store engines saturated from ~6us to the end.
  - Rows are processed in 128-row blocks (rows on partitions; remainder 64
    rows on partitions 0..63 — store descriptors are dealt evenly across
    the 16 SDMA engines only for partition counts 128/64 at base 0, so
    only those shapes are used). Histogram via iota-compare on DVE
    (tensor_scalar is_equal + scalar_tensor_tensor accumulate), then
    broadcast tensor_tensor multiplies produce [128, 2048] chunks =
    counts[:,n] * emb[n,d]; each chunk is a 1 MB DMA store (contiguous
    8 KB per partition).
  - HBM traffic is stores only: the 128-partition emb replica is NOT
    loaded from HBM (a 4 MB read = ~10 us of the ~140 us HBM budget; even
    1 MB = ~2.5 us). The whole replica is built on-chip by the otherwise
    idle TensorEngine: bf16 ones[1,128]^T @ emb1[1,512] outer products
    into PSUM, drained to SBUF by the otherwise idle ScalarE. Only a
    16 KB bf16 emb row is read from HBM. bf16 rounding of emb gives rel
    err ~2^-8, far inside the 2e-2 tolerance.
  - Block 0's chunk-0 multiply and store are split into 512-col pieces so
    the first store enters the queue as early as possible.
"""

import numpy as np
import ml_dtypes

import concourse.bass as bass
import concourse.mybir as mybir
from concourse import bacc
from concourse.tile import TileContext
from concourse.bass_utils import run_bass_kernel_spmd

B, S, K = 64, 200, 4
N, D = 128, 64
ND = N * D                      # 8192
NCORES = 8
B_LOC = B // NCORES             # 8
ROWS = B_LOC * S                # 1600 (b,s) rows per core
P = 128
NBLK = (ROWS + P - 1) // P      # 13 (12 full + 1 of 64 rows)

CH = 4                          # emb/mul/store chunks per block
CW = ND // CH                   # 2048 cols per chunk (= 32 n-rows), 1 MB stores
NCH = N // CH                   # 32 n-rows per chunk
MMW = 512                       # matmul moving-dim width (HW max)

_NC_CACHE = {}


def _build_nc():
    nc = bacc.Bacc()
    # first load: iota row (N cols) + block 0+1 indices, one small DMA
    iotx = nc.declare_dram_parameter("iotx", [P, N + K], mybir.dt.float32, isOutput=False)
    idx = nc.declare_dram_parameter("idx", [P, NBLK * K], mybir.dt.float32, isOutput=False)
    # emb row (bf16) with a trailing [1,128] row of ones for the matmul lhsT
    embone = nc.declare_dram_parameter("embone", [1, ND + P], mybir.dt.bfloat16, isOutput=False)
    out = nc.declare_dram_parameter("out", [ROWS, ND], mybir.dt.float32, isOutput=True)

    with TileContext(nc) as tc:
        with (
            tc.tile_pool(name="const", bufs=1) as cpool,
            tc.tile_pool(name="counts", bufs=NBLK) as hpool,
            tc.tile_pool(name="work", bufs=16) as wpool,
            tc.psum_pool(name="psum", bufs=4) as ppool,
        ):
            # iota + block-0 indices first (66 KB, one DMA): this alone
            # gates the first histogram
            iotx_sb = cpool.tile([P, N + K], mybir.dt.float32)
            nc.sync.dma_start(out=iotx_sb, in_=iotx[:, :])
            iota_sb = iotx_sb[:, 0:N]
            # emb row (+ ones for lhsT) next: it feeds the TensorE broadcast
            emb1_sb = cpool.tile([1, ND + P], mybir.dt.bfloat16)
            nc.sync.dma_start(out=emb1_sb, in_=embone[:, :])
            ones_sb = emb1_sb[:, ND : ND + P]
            # remaining indices (blocks 1..12)
            idx_sb = cpool.tile([P, NBLK * K], mybir.dt.float32)
            nc.sync.dma_start(out=idx_sb[:, K:], in_=idx[:, K:])

            # the 128-partition emb replica built by TensorE outer
            # products, drained PSUM->SBUF by ScalarE. Chunk 0 first: the
            # first multiplies gate on its 512-col slices (~11.5us; G0's
            # histograms fill that window on DVE).
            emb_sb = cpool.tile([P, ND], mybir.dt.float32)
            for c in range(CH):
                for s in range(CW // MMW):
                    col = c * CW + s * MMW
                    pt = ppool.tile([P, MMW], mybir.dt.float32, tag="pt")
                    nc.tensor.matmul(
                        pt[:, :],
                        lhsT=ones_sb[:, :],
                        rhs=emb1_sb[:, col : col + MMW],
                        start=True,
                        stop=True,
                    )
                    nc.scalar.copy(out=emb_sb[:, col : col + MMW], in_=pt[:, :])

            def emit_hist(j, counts, pj):
                # block 0's indices arrive with the iota in the first DMA
                src = iotx_sb if j == 0 else idx_sb
                col = N if j == 0 else j * K
                nc.vector.tensor_scalar(
                    out=counts[:pj],
                    in0=iota_sb[:pj],
                    scalar1=src[:pj, col : col + 1],
                    scalar2=None,
                    op0=mybir.AluOpType.is_equal,
                )
                for k in range(1, K):
                    nc.vector.scalar_tensor_tensor(
                        out=counts[:pj],
                        in0=iota_sb[:pj],
                        scalar=src[:pj, col + k : col + k + 1],
                        in1=counts[:pj],
                        op0=mybir.AluOpType.is_equal,
                        op1=mybir.AluOpType.add,
                    )

            def emit_mul(j, c, counts, pj, split=1):
                ot = wpool.tile([P, CW], mybir.dt.float32, tag="ot")
                w = CW // split
                nw = NCH // split
                for s in range(split):
                    emb_ap = emb_sb[:pj, c * CW + s * w : c * CW + (s + 1) * w]
                    nc.vector.tensor_tensor(
                        out=ot[:pj, s * w : (s + 1) * w].rearrange(
                            "p (n d) -> p n d", d=D
                        ),
                        in0=counts[
                            :pj, c * NCH + s * nw : c * NCH + (s + 1) * nw, None
                        ].broadcast_to([pj, nw, D]),
                        in1=emb_ap.rearrange("p (n d) -> p n d", d=D),
                        op=mybir.AluOpType.mult,
                    )
                    nc.sync.dma_start(
                        out=out[
                            j * P : j * P + pj, c * CW + s * w : c * CW + (s + 1) * w
                        ],
                        in_=ot[:pj, s * w : (s + 1) * w],
                    )

            # Blocks are processed in groups of 4: histogram + chunk-0
            # multiply for the group, then its chunks 1..3. Histograms cost
            # DVE time without producing store bytes; grouping spreads them
            # across the whole stream so DVE production (the store
            # producer) never drops below the DMA drain rate for long.
            # Group 0's chunk-1 multiplies start ~21us in, after TensorE
            # has replicated those emb columns (~13us).
            for g in range(0, NBLK, 4):
                blocks = range(g, min(g + 4, NBLK))
                counts_tiles = {}
                for j in blocks:
                    pj = min(P, ROWS - j * P)
                    counts = hpool.tile([P, N], mybir.dt.float32, tag="counts")
                    counts_tiles[j] = counts
                    emit_hist(j, counts, pj)
                for j in blocks:
                    pj = min(P, ROWS - j * P)
                    emit_mul(j, 0, counts_tiles[j], pj, split=4 if j == 0 else 1)
                for c in range(1, CH):
                    for j in blocks:
                        pj = min(P, ROWS - j * P)
                        emit_mul(j, c, counts_tiles[j], pj)

    nc.finalize()
    return nc


def _get_nc():
    if "nc" not in _NC_CACHE:
        _NC_CACHE["nc"] = _build_nc()
    return _NC_CACHE["nc"]


def _prepare_in_maps(concepts, emb_table):
    concepts = np.asarray(concepts)
    emb = np.ascontiguousarray(np.asarray(emb_table, dtype=np.float32).reshape(1, ND))

    # per-core index shards, padded to NBLK*P rows, laid out [P, NBLK*K]
    conc = concepts.reshape(NCORES, ROWS, K).astype(np.float32)
    idx_pad = np.full((NCORES, NBLK * P, K), float(N), dtype=np.float32)
    idx_pad[:, :ROWS] = conc
    # [core, NBLK, P, K] -> [core, P, NBLK*K]
    idx_dev = np.ascontiguousarray(
        idx_pad.reshape(NCORES, NBLK, P, K).transpose(0, 2, 1, 3).reshape(NCORES, P, NBLK * K)
    )

    embone = np.ascontiguousarray(
        np.concatenate(
            [emb.astype(ml_dtypes.bfloat16), np.ones((1, P), dtype=ml_dtypes.bfloat16)],
            axis=1,
        )
    )
    iota = np.broadcast_to(np.arange(N, dtype=np.float32), (NCORES, P, N))
    iotx = np.ascontiguousarray(
        np.concatenate([iota, idx_dev[:, :, 0:K]], axis=2)
    )
    return [
        {"iotx": iotx[i], "idx": idx_dev[i], "embone": embone}
        for i in range(NCORES)
    ]


def _run(concepts, emb_table, **spmd_kwargs):
    nc = _get_nc()
    in_maps = _prepare_in_maps(concepts, emb_table)
    res = run_bass_kernel_spmd(nc, in_maps, core_ids=list(range(NCORES)), **spmd_kwargs)
    out = np.concatenate(
        [res.results[i]["out"].reshape(B_LOC, S, N, D) for i in range(NCORES)],
        axis=0,
    )
    return out, res


def kernel(concepts, emb_table):
    out, _ = _run(concepts, emb_table)
    return out
